# revision 12
# baseline (speedup 1.0000x reference)
"""BM25 scoring kernel for Trainium2 (8 NeuronCores, SPMD).

score = sum_v term1(qtf_v) * term2(ptf_v) * term3(dfs_v)

term1 is nonzero only at the <=4096 query token ids, so instead of
materializing 8M-entry histograms we work query-position-centric:

  score = sum_i  term2(ptf[t_i]) * term3(dfs[t_i]) / (K3 + qtf[t_i])

where t_i ranges over all 4096 query positions (each unique id t appears
qtf_t times, and term1(q)/q = 1/(K3+q), so the sum telescopes exactly).

Sharding: query positions are split across the 8 cores (512 each, laid
out [128 partitions x 4 columns]).  Each core:
  - counts qtf (matches vs the full 4096-id query list) and ptf (matches
    vs the full 8192-id passage list).  The id lists arrive partition-
    broadcast in SBUF chunks; count units (chunk x column) are split
    between DVE (fused is_equal+rowsum TENSOR_SCALAR_CACHE_REDUCE, 1x
    mode) and ACT (Sign(x - q) then Square with row-sum accumulator,
    which yields chunk_len - count).
  - gathers dfs at its 512 ids with indirect (SWDGE) DMAs; these overlap
    the DVE compares, which is safe because 1x-mode DVE ops never take
    the shared SBUF port pair that gpsimd needs.
  - evaluates the BM25 terms on [128,4] tiles and reduces to one scalar
    (PE matmul against ones for the partition reduction).
Host stages the id lists as exact fp32 (values < 2^24) and sums the 8
per-core partials (the final all-reduce).
"""

import math
import os
from contextlib import ExitStack

import numpy as np

import concourse.bacc as bacc
import concourse.bass as bass
import concourse.tile as tile
from concourse import mybir
from concourse.bass_utils import run_bass_kernel_spmd

# ---- problem constants (from the BM25 reference) ----
VOCAB = 8_388_608
NQ = 4096
NP = 8192
K1, K3, B = 1.2, 8.0, 0.75
N_DOCS = 8_841_823.0
L_AVE = 55.0
L_D = NP  # passage length (static)
C2 = K1 * (1.0 - B + B * L_D / L_AVE)  # term2 denominator constant
INV_LN2 = 1.0 / math.log(2.0)

NCORES = 8
MYQ = NQ // NCORES  # 512 query positions per core
P = 128
QCOLS = MYQ // P  # 4 columns of [128]
CHUNK = 2048  # id-list chunk length (free dim of one compare op)
NQC = NQ // CHUNK  # 2 query-list chunks
NPC = NP // CHUNK  # 4 passage-list chunks

F32 = mybir.dt.float32
I32 = mybir.dt.int32

DBG_NO_GATHER = bool(int(os.environ.get("BM25_NO_GATHER", "0")))

# (chunk j, col k) count units: which go to ACT (Sign+Square) vs DVE
ACT_UNITS = frozenset(
    (j, k) for j in range(NQC + NPC) for k in range(QCOLS)
    if (4 * j + k) % 8 in (2, 5, 7)
)


def _build_program():
    nc = bacc.Bacc(
        "TRN2", target_bir_lowering=False, debug=False, num_devices=NCORES
    )
    qidsf = nc.dram_tensor("qidsf", [1, NQ], F32, kind="ExternalInput").ap()
    pidsf = nc.dram_tensor("pidsf", [1, NP], F32, kind="ExternalInput").ap()
    myq = nc.dram_tensor("myq", [P, QCOLS], I32, kind="ExternalInput").ap()
    myqf = nc.dram_tensor("myqf", [P, QCOLS], F32, kind="ExternalInput").ap()
    dfs = nc.dram_tensor("dfs", [VOCAB, 1], F32, kind="ExternalInput").ap()
    partial = nc.dram_tensor("partial", [1, 1], F32, kind="ExternalOutput").ap()

    with tile.TileContext(nc) as tc, ExitStack() as ctx:
        cpool = ctx.enter_context(tc.tile_pool(name="chunks", bufs=NQC + NPC))
        gpool = ctx.enter_context(tc.tile_pool(name="sgn", bufs=2))
        spool = ctx.enter_context(tc.tile_pool(name="small", bufs=1))
        dpool = ctx.enter_context(tc.tile_pool(name="dummy", bufs=1))
        ppool = ctx.enter_context(tc.tile_pool(name="psum", bufs=1, space="PSUM"))

        # small tiles initialized on gpsimd (its stream also owns the gather;
        # DVE must not run 2-port ops while gpsimd touches SBUF)
        bias_a = spool.tile([P, 1], F32)
        nc.gpsimd.memset(bias_a[:], float(N_DOCS + 0.5))
        bias_b = spool.tile([P, 1], F32)
        nc.gpsimd.memset(bias_b[:], 0.5)
        ones = spool.tile([P, 1], F32)
        nc.gpsimd.memset(ones[:], 1.0)
        part_q_d = spool.tile([P, QCOLS * NQC], F32)
        part_q_i = spool.tile([P, QCOLS * NQC], F32)
        part_p_d = spool.tile([P, QCOLS * NPC], F32)
        part_p_i = spool.tile([P, QCOLS * NPC], F32)
        for t in (part_q_d, part_q_i, part_p_d, part_p_i):
            nc.gpsimd.memset(t[:], 0.0)

        # my 512 query ids (int for the gather, f32 for the compares)
        myq_i = spool.tile([P, QCOLS], I32)
        nc.sync.dma_start(out=myq_i[:], in_=myq[:])
        myq_f = spool.tile([P, QCOLS], F32)
        nc.sync.dma_start(out=myq_f[:], in_=myqf[:])

        # dfs gather at my ids (SWDGE indirect DMA; one index per partition
        # per transfer -> one DMA per column).  Overlaps the 1x DVE compares.
        dfsg = spool.tile([P, QCOLS], F32)
        if DBG_NO_GATHER:
            nc.gpsimd.memset(dfsg[:], 500.0)
        else:
            for k in range(QCOLS):
                nc.gpsimd.indirect_dma_start(
                    out=dfsg[:, k : k + 1],
                    out_offset=None,
                    in_=dfs[:],
                    in_offset=bass.IndirectOffsetOnAxis(
                        ap=myq_i[:, k : k + 1], axis=0
                    ),
                )

        # ACT warm-up: load the Ln table set early; negated ids for Sign bias
        warm = spool.tile([P, 1], F32)
        nc.scalar.activation(
            warm[:], myq_f[:, 0:1], mybir.ActivationFunctionType.Ln,
            bias=bias_b[:],
        )
        negq = spool.tile([P, QCOLS], F32)
        nc.scalar.activation(
            negq[:], myq_f[:], mybir.ActivationFunctionType.Copy,
            bias=0.0, scale=-1.0,
        )

        # id-list broadcast loads (HWDGE), then the 24 count units
        def count_list(src_ap, nchunks, base_j, part_d, part_i, nper):
            for j in range(nchunks):
                ch = cpool.tile([P, CHUNK], F32, tag="chunk")
                bsrc = src_ap[
                    0:1, j * CHUNK : (j + 1) * CHUNK
                ].partition_broadcast(P)
                nc.sync.dma_start(out=ch[:], in_=bsrc)
                for k in range(QCOLS):
                    col = part_d[:, k * nper + j : k * nper + j + 1]
                    coli = part_i[:, k * nper + j : k * nper + j + 1]
                    if (base_j + j, k) in ACT_UNITS:
                        sgn = gpool.tile([P, CHUNK], F32, tag="sgn")
                        nc.scalar.activation(
                            sgn[:], ch[:], mybir.ActivationFunctionType.Sign,
                            bias=negq[:, k : k + 1], scale=1.0,
                        )
                        dummy2 = dpool.tile([P, CHUNK], F32, tag="dummy2")
                        nc.scalar.activation(
                            dummy2[:], sgn[:],
                            mybir.ActivationFunctionType.Square,
                            bias=0.0, scale=1.0, accum_out=coli,
                        )
                    else:
                        dummy = dpool.tile([P, CHUNK], F32, tag="dummy")
                        nc.vector.tensor_scalar(
                            out=dummy[:],
                            in0=ch[:],
                            scalar1=myq_f[:, k : k + 1],
                            scalar2=None,
                            op0=mybir.AluOpType.is_equal,
                            op1=mybir.AluOpType.add,
                            accum_out=col,
                        )

        count_list(qidsf, NQC, 0, part_q_d, part_q_i, NQC)
        count_list(pidsf, NPC, NQC, part_p_d, part_p_i, NPC)

        # combine partials -> qtf/ptf [128, 4]
        # count = sum(direct) + n_act*CHUNK - sum(inverted)
        def combine(part_d, part_i, nper, base_j, out_t):
            for k in range(QCOLS):
                n_act = sum(
                    1 for j in range(nper) if (base_j + j, k) in ACT_UNITS
                )
                dsum = spool.tile([P, 1], F32, tag=f"dsum{base_j}_{k}")
                nc.vector.tensor_reduce(
                    out=dsum[:], in_=part_d[:, k * nper : (k + 1) * nper],
                    axis=mybir.AxisListType.X, op=mybir.AluOpType.add,
                )
                isum = spool.tile([P, 1], F32, tag=f"isum{base_j}_{k}")
                nc.vector.tensor_reduce(
                    out=isum[:], in_=part_i[:, k * nper : (k + 1) * nper],
                    axis=mybir.AxisListType.X, op=mybir.AluOpType.add,
                )
                # (isum * -1 + n_act*CHUNK) + dsum
                nc.vector.tensor_scalar(
                    out=isum[:], in0=isum[:], scalar1=-1.0,
                    scalar2=float(n_act * CHUNK),
                    op0=mybir.AluOpType.mult, op1=mybir.AluOpType.add,
                )
                nc.vector.tensor_add(out_t[:, k : k + 1], dsum[:], isum[:])

        qtf = spool.tile([P, QCOLS], F32)
        ptf = spool.tile([P, QCOLS], F32)
        combine(part_q_d, part_q_i, NQC, 0, qtf)
        combine(part_p_d, part_p_i, NPC, NQC, ptf)

        # term1/qtf = 1/(K3 + qtf)
        ra = spool.tile([P, QCOLS], F32)
        nc.vector.tensor_scalar(
            out=ra[:], in0=qtf[:], scalar1=float(K3), scalar2=None,
            op0=mybir.AluOpType.add,
        )
        nc.vector.reciprocal(ra[:], ra[:])

        # term2 = K1 * ptf / (ptf + C2)   (exact 0 when ptf == 0)
        rb = spool.tile([P, QCOLS], F32)
        nc.vector.tensor_scalar(
            out=rb[:], in0=ptf[:], scalar1=float(C2), scalar2=None,
            op0=mybir.AluOpType.add,
        )
        nc.vector.reciprocal(rb[:], rb[:])
        t2 = spool.tile([P, QCOLS], F32)
        nc.vector.tensor_mul(t2[:], ptf[:], rb[:])

        # term3 = ln(N+0.5 - dfs) - ln(dfs + 0.5)   [log2 folded below]
        la = spool.tile([P, QCOLS], F32)
        nc.scalar.activation(
            la[:], dfsg[:], mybir.ActivationFunctionType.Ln,
            bias=bias_a[:], scale=-1.0,
        )
        lb = spool.tile([P, QCOLS], F32)
        nc.scalar.activation(
            lb[:], dfsg[:], mybir.ActivationFunctionType.Ln,
            bias=bias_b[:], scale=1.0,
        )
        t3 = spool.tile([P, QCOLS], F32)
        nc.vector.tensor_sub(t3[:], la[:], lb[:])

        # w = ra * t2 * t3, rowsum, fold K1/ln2
        w = spool.tile([P, QCOLS], F32)
        nc.vector.tensor_mul(w[:], ra[:], t2[:])
        w2 = spool.tile([P, QCOLS], F32)
        nc.vector.tensor_mul(w2[:], w[:], t3[:])
        rowsum = spool.tile([P, 1], F32)
        nc.vector.tensor_reduce(
            out=rowsum[:], in_=w2[:],
            axis=mybir.AxisListType.X, op=mybir.AluOpType.add,
        )
        nc.vector.tensor_scalar(
            out=rowsum[:], in0=rowsum[:], scalar1=float(K1 * INV_LN2),
            scalar2=None, op0=mybir.AluOpType.mult,
        )

        # partition reduce via matmul with ones
        acc = ppool.tile([1, 1], F32, space="PSUM")
        nc.tensor.matmul(acc[:], lhsT=rowsum[:], rhs=ones[:], start=True, stop=True)
        res = spool.tile([1, 1], F32)
        nc.vector.tensor_copy(res[:], acc[:])
        nc.sync.dma_start(out=partial[:], in_=res[:])

    nc.compile()
    return nc


_NC_CACHE = None


def _get_program():
    global _NC_CACHE
    if _NC_CACHE is None:
        _NC_CACHE = _build_program()
    return _NC_CACHE


def make_in_maps(query_ids, passage_ids, dfs):
    q = np.ascontiguousarray(query_ids.reshape(1, NQ).astype(np.int32))
    p = np.ascontiguousarray(passage_ids.reshape(1, NP).astype(np.int32))
    # exact fp32 staging of the ids (all values < 2^24)
    qf = q.astype(np.float32)
    pf = p.astype(np.float32)
    d = np.ascontiguousarray(dfs.reshape(VOCAB, 1).astype(np.float32))
    in_maps = []
    for c in range(NCORES):
        myq = np.ascontiguousarray(q[0, c * MYQ : (c + 1) * MYQ].reshape(P, QCOLS))
        in_maps.append({
            "qidsf": qf, "pidsf": pf, "myq": myq,
            "myqf": myq.astype(np.float32), "dfs": d,
        })
    return in_maps


def kernel(query_ids, passage_ids, dfs, **run_kwargs):
    nc = _get_program()
    in_maps = make_in_maps(query_ids, passage_ids, dfs)
    res = run_bass_kernel_spmd(nc, in_maps, core_ids=list(range(NCORES)), **run_kwargs)
    total = np.float32(sum(float(r["partial"][0, 0]) for r in res.results))
    out = np.array([total], dtype=np.float32)
    kernel.last_results = res
    return out
